# revision 4
# baseline (speedup 1.0000x reference)
"""CrossNonLocal2D kernel v3 for Trainium2, 8-way batch-parallel SPMD.

This execution path charges ~80-170us PER INSTRUCTION (a ~78us global
dispatch plus per-engine queue serialization; engines overlap).  The design
minimizes instruction count with the PE queue (matmuls) as the bottleneck:

PE (576): theta 16 + phi 16 + g 16 + ST 256 + PV 256 + outconv 16
ACT (64): exp over 4-bank PSUM groups [128, 4, 512] -> PT bf16
DVE (~80): bias-moves, in-place tree row-sums, reciprocal, normalize, epilogue
GPSIMD:   casting DMAs (f32->bf16 on load), partition_all_reduce row-sums
DMA:      whole-tensor loads, one dma_start_transpose for gT, 1 store/chunk

Math per core (batch element b):
  th = theta_w @ xt + tb [I,N]; ph = phi_w @ xo + pb; g = g_w @ xo (g_b folded)
  per n-chunk c of 512: ST[t] = ph[t]^T th[:,c] -> exp -> PT[:,t,:] (bf16)
                        PV += gT[t]^T-style accum (lhsT=gT[t], rhs=PT[t])
  rowsum via in-place DVE tree over t + gpsimd partition_all_reduce
  y = PV/rowsum; out = x_this + w_eff @ y + b_eff   (BN folded on host)
"""

import os
import sys
import time

import numpy as np

for _p in ("/opt/trn_rl_repo",):
    if os.path.isdir(_p) and _p not in sys.path:
        sys.path.insert(0, _p)

import ml_dtypes  # noqa: E402
import concourse.bacc as bacc  # noqa: E402
import concourse.bass_isa as bass_isa  # noqa: E402
import concourse.mybir as mybir  # noqa: E402
import concourse.tile as tile  # noqa: E402
from concourse.bass import ts  # noqa: E402
from concourse.bass_utils import run_bass_kernel_spmd  # noqa: E402

B, C, HH, WW = 8, 256, 64, 64
N = HH * WW  # 4096
I = 128
NCORES = 8
BN_EPS = 1e-5
MT = N // 128  # 32 m-tiles
NCH = N // 512  # 8 n-chunks

f32 = mybir.dt.float32
bf16 = mybir.dt.bfloat16
EXP = mybir.ActivationFunctionType.Exp
ADD = mybir.AluOpType.add
MULT = mybir.AluOpType.mult
DIV = mybir.AluOpType.divide


def build_module(repeat: int = 1):
    nc = bacc.Bacc("TRN2", target_bir_lowering=False, debug=False,
                   num_devices=NCORES)

    xt_d = nc.dram_tensor("xt", [C, N], f32, kind="ExternalInput")
    xtp_d = nc.dram_tensor("xtp", [C, N], f32, kind="ExternalInput")
    xo_d = nc.dram_tensor("xo", [C, N], f32, kind="ExternalInput")
    # packed bf16 weights [128, 8, 128]: thwT(2) phwT(2) gwT(2) weffT(2)
    wb_d = nc.dram_tensor("wb", [128, 8, 128], bf16, kind="ExternalInput")
    # packed f32 scalars [128, 4]: tb pb beff0 beff1
    fp_d = nc.dram_tensor("fp", [128, 4], f32, kind="ExternalInput")
    out_d = nc.dram_tensor("out", [C, N], f32, kind="ExternalOutput")

    xt_v = xt_d.ap().rearrange("(a p) n -> p a n", p=128)
    xtp_v = xtp_d.ap().rearrange("(a p) n -> p a n", p=128)
    xo_v = xo_d.ap().rearrange("(a p) n -> p a n", p=128)
    out_v = out_d.ap().rearrange("(a p) n -> p a n", p=128)

    with tile.TileContext(nc) as tc:
        with (
            tc.tile_pool(name="const", bufs=1) as constp,
            tc.tile_pool(name="persist", bufs=1) as persist,
            tc.tile_pool(name="small", bufs=2) as smallp,
            tc.tile_pool(name="outb", bufs=2) as outbp,
            tc.tile_pool(name="pst", bufs=2, space="PSUM") as pst,   # 4 banks
            tc.tile_pool(name="ppv", bufs=1, space="PSUM") as ppv,   # 1 bank
            tc.tile_pool(name="poc", bufs=1, space="PSUM") as poc,   # 2 banks
        ):
            wb = constp.tile([128, 8, 128], bf16, tag="wb")
            nc.sync.dma_start(out=wb, in_=wb_d.ap())
            fp = constp.tile([128, 4], f32, tag="fp")
            nc.sync.dma_start(out=fp, in_=fp_d.ap())

            for _rep in range(repeat):
                # ---- loads (gpsimd DMAs cast f32->bf16 in flight) ----
                xtf = persist.tile([128, 2, N], f32, tag="xtf", name="xtf")
                nc.sync.dma_start(out=xtf, in_=xtp_v)
                xtb = persist.tile([128, 2, N], bf16, tag="xtb", name="xtb")
                nc.gpsimd.dma_start(out=xtb, in_=xt_v)
                xob = persist.tile([128, 2, N], bf16, tag="xob", name="xob")
                nc.gpsimd.dma_start(out=xob, in_=xo_v)

                th = persist.tile([128, NCH, 512], bf16, tag="th", name="th")
                ph = persist.tile([128, NCH, 512], bf16, tag="ph", name="ph")
                gsb = persist.tile([128, NCH, 512], bf16, tag="gsb", name="gsb")
                gT = persist.tile([128, MT, 128], bf16, tag="gT", name="gT")

                # ---- 1x1 convs: 4-bank PSUM groups, one DVE move per group ----
                for (src, w0, bias, dst) in (
                    (xtb, 0, 0, th), (xob, 2, 1, ph), (xob, 4, None, gsb),
                ):
                    for grp in range(4):
                        cv = pst.tile([128, 2, 512], f32, tag="st", name="cv")
                        for k in range(2):
                            s = 2 * grp + k
                            for a in range(2):
                                nc.tensor.matmul(cv[:, k, :],
                                                 lhsT=wb[:, w0 + a, :],
                                                 rhs=src[:, a, ts(s, 512)],
                                                 start=(a == 0), stop=(a == 1))
                        if bias is None:
                            nc.vector.tensor_copy(
                                dst[:, 2 * grp:2 * grp + 2, :], cv[:])
                        else:
                            nc.vector.tensor_scalar_add(
                                dst[:, 2 * grp:2 * grp + 2, :], cv[:],
                                fp[:, bias:bias + 1])

                # ---- gT = g^T via one transpose DMA ----
                nc.sync.dma_start_transpose(gT[:], gsb[:])

                # ---- attention, one n-chunk (512 cols) at a time ----
                PT = persist.tile([128, MT, 512], bf16, tag="PT", name="PT")
                ob = persist.tile([128, 2, NCH, 512], f32, tag="ob", name="ob")
                for c in range(NCH):
                    for q in range(16):
                        st = pst.tile([128, 2, 512], f32, tag="st",
                                      name=f"st{c}_{q}")
                        for k in range(2):
                            t = 2 * q + k
                            nc.tensor.matmul(st[:, k, :],
                                             lhsT=ph[:, t // 4, ts(t % 4, 128)],
                                             rhs=th[:, c, :],
                                             start=True, stop=True)
                        nc.scalar.activation(PT[:, 2 * q:2 * q + 2, :], st[:],
                                             EXP)
                    pv = ppv.tile([128, 512], f32, tag="pv", name=f"pv{c}")
                    for t in range(MT):
                        nc.tensor.matmul(pv[:], lhsT=gT[:, t, :],
                                         rhs=PT[:, t, :],
                                         start=(t == 0), stop=(t == MT - 1))
                    # in-place bf16 tree-sum over t (PT consumed by PV already)
                    h = MT // 2
                    while h >= 2:
                        nc.vector.tensor_tensor(PT[:, 0:h, :], PT[:, 0:h, :],
                                                PT[:, h:2 * h, :], op=ADD)
                        h //= 2
                    rs = smallp.tile([128, 512], f32, tag="rs")
                    nc.vector.tensor_tensor(rs[:], PT[:, 0, :], PT[:, 1, :],
                                            op=ADD)
                    rbc = smallp.tile([128, 512], f32, tag="rbc")
                    nc.gpsimd.partition_all_reduce(
                        rbc[:], rs[:], channels=128,
                        reduce_op=bass_isa.ReduceOp.add)
                    rinv = smallp.tile([128, 512], f32, tag="rinv")
                    nc.vector.reciprocal(rinv[:], rbc[:])
                    y_n = smallp.tile([128, 512], bf16, tag="yn")
                    nc.vector.tensor_tensor(y_n[:], pv[:], rinv[:], op=MULT)
                    # out conv + residual epilogue (b_eff pre-added into xtp)
                    oc = poc.tile([128, 2, 512], f32, tag="oc", name=f"oc{c}")
                    for hh in range(2):
                        nc.tensor.matmul(oc[:, hh, :], lhsT=wb[:, 6 + hh, :],
                                         rhs=y_n[:], start=True, stop=True)
                    nc.vector.tensor_tensor(ob[:, :, c, :], oc[:],
                                            xtf[:, :, ts(c, 512)], op=ADD)
                nc.sync.dma_start(out=out_v, in_=ob[:])

    nc.compile()
    return nc


_CACHE: dict = {}


def _get_built(repeat: int = 1):
    if repeat not in _CACHE:
        _CACHE[repeat] = build_module(repeat)
    return _CACHE[repeat]


def prep_maps(inputs: dict) -> list[dict]:
    """Host-side precompute: fold BN + g/out biases, pack weights."""
    f = lambda k: np.asarray(inputs[k], np.float32)
    x_this = f("x_this").reshape(B, C, N)
    x_other = f("x_other").reshape(B, C, N)
    theta_w, theta_b = f("theta_w"), f("theta_b")
    phi_w, phi_b = f("phi_w"), f("phi_b")
    g_w, g_b = f("g_w"), f("g_b")
    out_w, out_b = f("out_w"), f("out_b")
    gam, bet = f("bn_gamma"), f("bn_beta")
    mean, var = f("bn_mean"), f("bn_var")

    s = (gam / np.sqrt(var + BN_EPS)).astype(np.float32)  # [C]
    w_eff = (out_w * s[:, None]).astype(np.float32)  # [C, I]
    b_eff = (s * (out_w @ g_b + out_b - mean) + bet).astype(np.float32)  # [C]

    bf = ml_dtypes.bfloat16
    wb = np.zeros((128, 8, 128), dtype=bf)
    thwT = np.ascontiguousarray(theta_w.T).reshape(2, 128, I)  # [a, p, i]
    phwT = np.ascontiguousarray(phi_w.T).reshape(2, 128, I)
    gwT = np.ascontiguousarray(g_w.T).reshape(2, 128, I)
    weffT = np.ascontiguousarray(w_eff.T).reshape(I, 2, 128)  # [i, h, c]
    for a in range(2):
        wb[:, 0 + a, :] = thwT[a].astype(bf)
        wb[:, 2 + a, :] = phwT[a].astype(bf)
        wb[:, 4 + a, :] = gwT[a].astype(bf)
        wb[:, 6 + a, :] = weffT[:, a, :].astype(bf)

    fp = np.zeros((128, 4), dtype=np.float32)
    fp[:, 0] = theta_b
    fp[:, 1] = phi_b
    be = b_eff.reshape(2, 128)
    fp[:, 2] = be[0]
    fp[:, 3] = be[1]

    common = {"wb": wb, "fp": fp}
    return [
        {"xt": np.ascontiguousarray(x_this[b]),
         "xtp": np.ascontiguousarray(x_this[b] + b_eff[:, None]),
         "xo": np.ascontiguousarray(x_other[b]), **common}
        for b in range(B)
    ]


def run(inputs: dict, repeat: int = 1, time_it: bool = False):
    nc = _get_built(repeat)
    maps = prep_maps(inputs)
    t0 = time.time()
    res = run_bass_kernel_spmd(nc, maps, list(range(NCORES)))
    wall = time.time() - t0
    out = np.stack([np.asarray(res.results[b]["out"], np.float32)
                    for b in range(B)])
    out = out.reshape(B, C, HH, WW)
    if time_it:
        return out, wall
    return out


def kernel(**inputs) -> np.ndarray:
    return run(inputs)


# revision 5
# speedup vs baseline: 2.3531x; 2.3531x over previous
"""CrossNonLocal2D kernel v3 for Trainium2, 8-way batch-parallel SPMD.

This execution path charges ~80-170us PER INSTRUCTION (a ~78us global
dispatch plus per-engine queue serialization; engines overlap).  The design
minimizes instruction count with the PE queue (matmuls) as the bottleneck:

PE (576): theta 16 + phi 16 + g 16 + ST 256 + PV 256 + outconv 16
ACT (64): exp over 4-bank PSUM groups [128, 4, 512] -> PT bf16
DVE (~80): bias-moves, in-place tree row-sums, reciprocal, normalize, epilogue
GPSIMD:   casting DMAs (f32->bf16 on load), partition_all_reduce row-sums
DMA:      whole-tensor loads, one dma_start_transpose for gT, 1 store/chunk

Math per core (batch element b):
  th = theta_w @ xt + tb [I,N]; ph = phi_w @ xo + pb; g = g_w @ xo (g_b folded)
  per n-chunk c of 512: ST[t] = ph[t]^T th[:,c] -> exp -> PT[:,t,:] (bf16)
                        PV += gT[t]^T-style accum (lhsT=gT[t], rhs=PT[t])
  rowsum via in-place DVE tree over t + gpsimd partition_all_reduce
  y = PV/rowsum; out = x_this + w_eff @ y + b_eff   (BN folded on host)
"""

import os
import sys
import time

import numpy as np

for _p in ("/opt/trn_rl_repo",):
    if os.path.isdir(_p) and _p not in sys.path:
        sys.path.insert(0, _p)

import ml_dtypes  # noqa: E402
import concourse.bacc as bacc  # noqa: E402
import concourse.bass_isa as bass_isa  # noqa: E402
import concourse.mybir as mybir  # noqa: E402
import concourse.tile as tile  # noqa: E402
from concourse.bass import ts  # noqa: E402
from concourse.bass_utils import run_bass_kernel_spmd  # noqa: E402

B, C, HH, WW = 8, 256, 64, 64
N = HH * WW  # 4096
I = 128
NCORES = 8
BN_EPS = 1e-5
MT = N // 128  # 32 m-tiles
NCH = N // 512  # 8 n-chunks

f32 = mybir.dt.float32
bf16 = mybir.dt.bfloat16
EXP = mybir.ActivationFunctionType.Exp
ADD = mybir.AluOpType.add
MULT = mybir.AluOpType.mult
DIV = mybir.AluOpType.divide


def build_module(repeat: int = 1):
    nc = bacc.Bacc("TRN2", target_bir_lowering=False, debug=False,
                   num_devices=NCORES)

    xt_d = nc.dram_tensor("xt", [C, N], f32, kind="ExternalInput")
    xtp_d = nc.dram_tensor("xtp", [C, N], f32, kind="ExternalInput")
    xo_d = nc.dram_tensor("xo", [C, N], f32, kind="ExternalInput")
    # packed bf16 weights [128, 8, 128]: thwT(2) phwT(2) gwT(2) weffT(2)
    wb_d = nc.dram_tensor("wb", [128, 8, 128], bf16, kind="ExternalInput")
    # packed f32 scalars [128, 4]: tb pb beff0 beff1
    fp_d = nc.dram_tensor("fp", [128, 4], f32, kind="ExternalInput")
    out_d = nc.dram_tensor("out", [C, N], f32, kind="ExternalOutput")

    xt_v = xt_d.ap().rearrange("(a p) n -> p a n", p=128)
    xtp_v = xtp_d.ap().rearrange("(a p) n -> p a n", p=128)
    xo_v = xo_d.ap().rearrange("(a p) n -> p a n", p=128)
    out_v = out_d.ap().rearrange("(a p) n -> p a n", p=128)

    with tile.TileContext(nc) as tc:
        with (
            tc.tile_pool(name="const", bufs=1) as constp,
            tc.tile_pool(name="persist", bufs=1) as persist,
            tc.tile_pool(name="small", bufs=2) as smallp,
            tc.tile_pool(name="outb", bufs=2) as outbp,
            tc.tile_pool(name="pst", bufs=2, space="PSUM") as pst,   # 4 banks
            tc.tile_pool(name="ppv", bufs=1, space="PSUM") as ppv,   # 1 bank
            tc.tile_pool(name="poc", bufs=1, space="PSUM") as poc,   # 2 banks
        ):
            wb = constp.tile([128, 8, 128], bf16, tag="wb")
            nc.sync.dma_start(out=wb, in_=wb_d.ap())
            fp = constp.tile([128, 4], f32, tag="fp")
            nc.sync.dma_start(out=fp, in_=fp_d.ap())

            for _rep in range(repeat):
                # ---- loads (gpsimd DMAs cast f32->bf16 in flight) ----
                xtf = persist.tile([128, 2, N], f32, tag="xtf", name="xtf")
                nc.sync.dma_start(out=xtf, in_=xtp_v)
                xtb = persist.tile([128, 2, N], bf16, tag="xtb", name="xtb")
                nc.gpsimd.dma_start(out=xtb, in_=xt_v)
                xob = persist.tile([128, 2, N], bf16, tag="xob", name="xob")
                nc.gpsimd.dma_start(out=xob, in_=xo_v)

                th = persist.tile([128, NCH, 512], bf16, tag="th", name="th")
                ph = persist.tile([128, NCH, 512], bf16, tag="ph", name="ph")
                gsb = persist.tile([128, NCH, 512], bf16, tag="gsb", name="gsb")
                gT = persist.tile([128, MT, 128], bf16, tag="gT", name="gT")

                # ---- 1x1 convs: 4-bank PSUM groups, one DVE move per group ----
                for (src, w0, bias, dst) in (
                    (xtb, 0, 0, th), (xob, 2, 1, ph), (xob, 4, None, gsb),
                ):
                    for grp in range(4):
                        cv = pst.tile([128, 2, 512], f32, tag="st", name="cv")
                        for k in range(2):
                            s = 2 * grp + k
                            for a in range(2):
                                nc.tensor.matmul(cv[:, k, :],
                                                 lhsT=wb[:, w0 + a, :],
                                                 rhs=src[:, a, ts(s, 512)],
                                                 start=(a == 0), stop=(a == 1))
                        if bias is None:
                            nc.vector.tensor_copy(
                                dst[:, 2 * grp:2 * grp + 2, :], cv[:])
                        else:
                            nc.vector.tensor_scalar_add(
                                dst[:, 2 * grp:2 * grp + 2, :], cv[:],
                                fp[:, bias:bias + 1])

                # ---- gT = g^T via one transpose DMA ----
                nc.sync.dma_start_transpose(gT[:], gsb[:])

                # ---- attention, one n-chunk (512 cols) at a time ----
                PT = persist.tile([128, MT, 512], bf16, tag="PT", name="PT")
                for c in range(NCH):
                    for q in range(16):
                        st = pst.tile([128, 2, 512], f32, tag="st",
                                      name=f"st{c}_{q}")
                        for k in range(2):
                            t = 2 * q + k
                            nc.tensor.matmul(st[:, k, :],
                                             lhsT=ph[:, t // 4, ts(t % 4, 128)],
                                             rhs=th[:, c, :],
                                             start=True, stop=True)
                        nc.scalar.activation(PT[:, 2 * q:2 * q + 2, :], st[:],
                                             EXP)
                    pv = ppv.tile([128, 512], f32, tag="pv", name=f"pv{c}")
                    for t in range(MT):
                        nc.tensor.matmul(pv[:], lhsT=gT[:, t, :],
                                         rhs=PT[:, t, :],
                                         start=(t == 0), stop=(t == MT - 1))
                    # in-place bf16 tree-sum over t (PT consumed by PV already)
                    h = MT // 2
                    while h >= 2:
                        nc.vector.tensor_tensor(PT[:, 0:h, :], PT[:, 0:h, :],
                                                PT[:, h:2 * h, :], op=ADD)
                        h //= 2
                    rs = smallp.tile([128, 512], f32, tag="rs")
                    nc.vector.tensor_tensor(rs[:], PT[:, 0, :], PT[:, 1, :],
                                            op=ADD)
                    rbc = smallp.tile([128, 512], f32, tag="rbc")
                    nc.gpsimd.partition_all_reduce(
                        rbc[:], rs[:], channels=128,
                        reduce_op=bass_isa.ReduceOp.add)
                    rinv = smallp.tile([128, 512], f32, tag="rinv")
                    nc.vector.reciprocal(rinv[:], rbc[:])
                    y_n = smallp.tile([128, 512], bf16, tag="yn")
                    nc.vector.tensor_tensor(y_n[:], pv[:], rinv[:], op=MULT)
                    # out conv + residual epilogue (b_eff pre-added into xtp)
                    oc = poc.tile([128, 2, 512], f32, tag="oc", name=f"oc{c}")
                    for hh in range(2):
                        nc.tensor.matmul(oc[:, hh, :], lhsT=wb[:, 6 + hh, :],
                                         rhs=y_n[:], start=True, stop=True)
                    ob = outbp.tile([128, 2, 512], f32, tag="ob")
                    nc.vector.tensor_tensor(ob[:], oc[:],
                                            xtf[:, :, ts(c, 512)], op=ADD)
                    nc.sync.dma_start(out=out_v[:, :, ts(c, 512)], in_=ob[:])

    nc.compile()
    return nc


_CACHE: dict = {}


def _get_built(repeat: int = 1):
    if repeat not in _CACHE:
        _CACHE[repeat] = build_module(repeat)
    return _CACHE[repeat]


def prep_maps(inputs: dict) -> list[dict]:
    """Host-side precompute: fold BN + g/out biases, pack weights."""
    f = lambda k: np.asarray(inputs[k], np.float32)
    x_this = f("x_this").reshape(B, C, N)
    x_other = f("x_other").reshape(B, C, N)
    theta_w, theta_b = f("theta_w"), f("theta_b")
    phi_w, phi_b = f("phi_w"), f("phi_b")
    g_w, g_b = f("g_w"), f("g_b")
    out_w, out_b = f("out_w"), f("out_b")
    gam, bet = f("bn_gamma"), f("bn_beta")
    mean, var = f("bn_mean"), f("bn_var")

    s = (gam / np.sqrt(var + BN_EPS)).astype(np.float32)  # [C]
    w_eff = (out_w * s[:, None]).astype(np.float32)  # [C, I]
    b_eff = (s * (out_w @ g_b + out_b - mean) + bet).astype(np.float32)  # [C]

    bf = ml_dtypes.bfloat16
    wb = np.zeros((128, 8, 128), dtype=bf)
    thwT = np.ascontiguousarray(theta_w.T).reshape(2, 128, I)  # [a, p, i]
    phwT = np.ascontiguousarray(phi_w.T).reshape(2, 128, I)
    gwT = np.ascontiguousarray(g_w.T).reshape(2, 128, I)
    weffT = np.ascontiguousarray(w_eff.T).reshape(I, 2, 128)  # [i, h, c]
    for a in range(2):
        wb[:, 0 + a, :] = thwT[a].astype(bf)
        wb[:, 2 + a, :] = phwT[a].astype(bf)
        wb[:, 4 + a, :] = gwT[a].astype(bf)
        wb[:, 6 + a, :] = weffT[:, a, :].astype(bf)

    fp = np.zeros((128, 4), dtype=np.float32)
    fp[:, 0] = theta_b
    fp[:, 1] = phi_b
    be = b_eff.reshape(2, 128)
    fp[:, 2] = be[0]
    fp[:, 3] = be[1]

    common = {"wb": wb, "fp": fp}
    return [
        {"xt": np.ascontiguousarray(x_this[b]),
         "xtp": np.ascontiguousarray(x_this[b] + b_eff[:, None]),
         "xo": np.ascontiguousarray(x_other[b]), **common}
        for b in range(B)
    ]


def run(inputs: dict, repeat: int = 1, time_it: bool = False):
    nc = _get_built(repeat)
    maps = prep_maps(inputs)
    t0 = time.time()
    res = run_bass_kernel_spmd(nc, maps, list(range(NCORES)))
    wall = time.time() - t0
    out = np.stack([np.asarray(res.results[b]["out"], np.float32)
                    for b in range(B)])
    out = out.reshape(B, C, HH, WW)
    if time_it:
        return out, wall
    return out


def kernel(**inputs) -> np.ndarray:
    return run(inputs)


# revision 6
# speedup vs baseline: 2.6824x; 1.1400x over previous
"""CrossNonLocal2D kernel v3 for Trainium2, 8-way batch-parallel SPMD.

This execution path charges ~80-170us PER INSTRUCTION (a ~78us global
dispatch plus per-engine queue serialization; engines overlap).  The design
minimizes instruction count with the PE queue (matmuls) as the bottleneck:

PE (576): theta 16 + phi 16 + g 16 + ST 256 + PV 256 + outconv 16
ACT (64): exp over 4-bank PSUM groups [128, 4, 512] -> PT bf16
DVE (~80): bias-moves, in-place tree row-sums, reciprocal, normalize, epilogue
GPSIMD:   casting DMAs (f32->bf16 on load), partition_all_reduce row-sums
DMA:      whole-tensor loads, one dma_start_transpose for gT, 1 store/chunk

Math per core (batch element b):
  th = theta_w @ xt + tb [I,N]; ph = phi_w @ xo + pb; g = g_w @ xo (g_b folded)
  per n-chunk c of 512: ST[t] = ph[t]^T th[:,c] -> exp -> PT[:,t,:] (bf16)
                        PV += gT[t]^T-style accum (lhsT=gT[t], rhs=PT[t])
  rowsum via in-place DVE tree over t + gpsimd partition_all_reduce
  y = PV/rowsum; out = x_this + w_eff @ y + b_eff   (BN folded on host)
"""

import os
import sys
import time

import numpy as np

for _p in ("/opt/trn_rl_repo",):
    if os.path.isdir(_p) and _p not in sys.path:
        sys.path.insert(0, _p)

import ml_dtypes  # noqa: E402
import concourse.bacc as bacc  # noqa: E402
import concourse.bass_isa as bass_isa  # noqa: E402
import concourse.mybir as mybir  # noqa: E402
import concourse.tile as tile  # noqa: E402
from concourse.bass import ts  # noqa: E402
from concourse.bass_utils import run_bass_kernel_spmd  # noqa: E402

B, C, HH, WW = 8, 256, 64, 64
N = HH * WW  # 4096
I = 128
NCORES = 8
BN_EPS = 1e-5
MT = N // 128  # 32 m-tiles
NCH = N // 512  # 8 n-chunks

f32 = mybir.dt.float32
bf16 = mybir.dt.bfloat16
EXP = mybir.ActivationFunctionType.Exp
ADD = mybir.AluOpType.add
MULT = mybir.AluOpType.mult
DIV = mybir.AluOpType.divide


def build_module(repeat: int = 1):
    nc = bacc.Bacc("TRN2", target_bir_lowering=False, debug=False,
                   num_devices=NCORES)

    xt_d = nc.dram_tensor("xt", [C, N], f32, kind="ExternalInput")
    xtp_d = nc.dram_tensor("xtp", [C, N], f32, kind="ExternalInput")
    xo_d = nc.dram_tensor("xo", [C, N], f32, kind="ExternalInput")
    # packed bf16 weights [128, 8, 128]: thwT(2) phwT(2) gwT(2) weffT(2)
    wb_d = nc.dram_tensor("wb", [128, 8, 128], bf16, kind="ExternalInput")
    # packed f32 scalars [128, 4]: tb pb beff0 beff1
    fp_d = nc.dram_tensor("fp", [128, 4], f32, kind="ExternalInput")
    out_d = nc.dram_tensor("out", [C, N], f32, kind="ExternalOutput")

    xt_v = xt_d.ap().rearrange("(a p) n -> p a n", p=128)
    xtp_v = xtp_d.ap().rearrange("(a p) n -> p a n", p=128)
    xo_v = xo_d.ap().rearrange("(a p) n -> p a n", p=128)
    out_v = out_d.ap().rearrange("(a p) n -> p a n", p=128)

    with tile.TileContext(nc) as tc:
        with (
            tc.tile_pool(name="const", bufs=1) as constp,
            tc.tile_pool(name="persist", bufs=1) as persist,
            tc.tile_pool(name="small", bufs=2) as smallp,
            tc.tile_pool(name="outb", bufs=2) as outbp,
            tc.tile_pool(name="pst", bufs=2, space="PSUM") as pst,   # 4 banks
            tc.tile_pool(name="ppv", bufs=1, space="PSUM") as ppv,   # 1 bank
            tc.tile_pool(name="poc", bufs=1, space="PSUM") as poc,   # 2 banks
        ):
            wb = constp.tile([128, 8, 128], bf16, tag="wb")
            nc.sync.dma_start(out=wb, in_=wb_d.ap())
            fp = constp.tile([128, 4], f32, tag="fp")
            nc.sync.dma_start(out=fp, in_=fp_d.ap())

            for _rep in range(repeat):
                # ---- loads (gpsimd DMAs cast f32->bf16 in flight) ----
                xtf = persist.tile([128, 2, N], f32, tag="xtf", name="xtf")
                nc.sync.dma_start(out=xtf, in_=xtp_v)
                xtb = persist.tile([128, 2, N], bf16, tag="xtb", name="xtb")
                nc.gpsimd.dma_start(out=xtb, in_=xt_v)
                xob = persist.tile([128, 2, N], bf16, tag="xob", name="xob")
                nc.gpsimd.dma_start(out=xob, in_=xo_v)

                th = persist.tile([128, NCH, 512], bf16, tag="th", name="th")
                ph = persist.tile([128, NCH, 512], bf16, tag="ph", name="ph")
                gsb = persist.tile([128, NCH, 512], bf16, tag="gsb", name="gsb")
                gT = persist.tile([128, MT, 128], bf16, tag="gT", name="gT")

                # ---- 1x1 convs: 4-bank PSUM groups, one DVE move per group ----
                for (src, w0, bias, dst) in (
                    (xtb, 0, 0, th), (xob, 2, 1, ph), (xob, 4, None, gsb),
                ):
                    for grp in range(4):
                        cv = pst.tile([128, 2, 512], f32, tag="st", name="cv")
                        for k in range(2):
                            s = 2 * grp + k
                            for a in range(2):
                                nc.tensor.matmul(cv[:, k, :],
                                                 lhsT=wb[:, w0 + a, :],
                                                 rhs=src[:, a, ts(s, 512)],
                                                 start=(a == 0), stop=(a == 1))
                        if bias is None:
                            nc.vector.tensor_copy(
                                dst[:, 2 * grp:2 * grp + 2, :], cv[:])
                        else:
                            nc.vector.tensor_scalar_add(
                                dst[:, 2 * grp:2 * grp + 2, :], cv[:],
                                fp[:, bias:bias + 1])

                # ---- gT = g^T via one transpose DMA ----
                nc.sync.dma_start_transpose(gT[:], gsb[:])

                # ---- attention: chunk-PAIRS share one softmax epilogue ----
                PT = persist.tile([128, MT, 2, 512], bf16, tag="PT", name="PT")
                for cp in range(NCH // 2):
                    for cc in range(2):
                        c = 2 * cp + cc
                        for q in range(16):
                            st = pst.tile([128, 2, 512], f32, tag="st",
                                          name=f"st{c}_{q}")
                            for k in range(2):
                                t = 2 * q + k
                                nc.tensor.matmul(st[:, k, :],
                                                 lhsT=ph[:, t // 4, ts(t % 4, 128)],
                                                 rhs=th[:, c, :],
                                                 start=True, stop=True)
                            nc.scalar.activation(PT[:, 2 * q:2 * q + 2, cc, :],
                                                 st[:], EXP)
                    pv = ppv.tile([128, 2, 512], f32, tag="pv", name=f"pv{cp}")
                    for cc in range(2):
                        for t in range(MT):
                            nc.tensor.matmul(pv[:, cc, :], lhsT=gT[:, t, :],
                                             rhs=PT[:, t, cc, :],
                                             start=(t == 0), stop=(t == MT - 1))
                    # in-place bf16 tree-sum over t for BOTH chunks at once
                    h = MT // 2
                    while h >= 2:
                        nc.vector.tensor_tensor(PT[:, 0:h, :, :], PT[:, 0:h, :, :],
                                                PT[:, h:2 * h, :, :], op=ADD)
                        h //= 2
                    rs = smallp.tile([128, 2, 512], f32, tag="rs")
                    nc.vector.tensor_tensor(rs[:], PT[:, 0, :, :], PT[:, 1, :, :],
                                            op=ADD)
                    rbc = smallp.tile([128, 2, 512], f32, tag="rbc")
                    nc.gpsimd.partition_all_reduce(
                        rbc[:], rs[:], channels=128,
                        reduce_op=bass_isa.ReduceOp.add)
                    rinv = smallp.tile([128, 2, 512], f32, tag="rinv")
                    nc.vector.reciprocal(rinv[:], rbc[:])
                    y_n = smallp.tile([128, 2, 512], bf16, tag="yn")
                    nc.vector.tensor_tensor(y_n[:], pv[:], rinv[:], op=MULT)
                    # out conv + residual epilogue (b_eff pre-added into xtp)
                    for cc in range(2):
                        c = 2 * cp + cc
                        oc = poc.tile([128, 2, 512], f32, tag="oc", name=f"oc{c}")
                        for hh in range(2):
                            nc.tensor.matmul(oc[:, hh, :], lhsT=wb[:, 6 + hh, :],
                                             rhs=y_n[:, cc, :],
                                             start=True, stop=True)
                        ob = outbp.tile([128, 2, 512], f32, tag="ob")
                        nc.vector.tensor_tensor(ob[:], oc[:],
                                                xtf[:, :, ts(c, 512)], op=ADD)
                        nc.sync.dma_start(out=out_v[:, :, ts(c, 512)], in_=ob[:])

    nc.compile()
    return nc


_CACHE: dict = {}


def _get_built(repeat: int = 1):
    if repeat not in _CACHE:
        _CACHE[repeat] = build_module(repeat)
    return _CACHE[repeat]


def prep_maps(inputs: dict) -> list[dict]:
    """Host-side precompute: fold BN + g/out biases, pack weights."""
    f = lambda k: np.asarray(inputs[k], np.float32)
    x_this = f("x_this").reshape(B, C, N)
    x_other = f("x_other").reshape(B, C, N)
    theta_w, theta_b = f("theta_w"), f("theta_b")
    phi_w, phi_b = f("phi_w"), f("phi_b")
    g_w, g_b = f("g_w"), f("g_b")
    out_w, out_b = f("out_w"), f("out_b")
    gam, bet = f("bn_gamma"), f("bn_beta")
    mean, var = f("bn_mean"), f("bn_var")

    s = (gam / np.sqrt(var + BN_EPS)).astype(np.float32)  # [C]
    w_eff = (out_w * s[:, None]).astype(np.float32)  # [C, I]
    b_eff = (s * (out_w @ g_b + out_b - mean) + bet).astype(np.float32)  # [C]

    bf = ml_dtypes.bfloat16
    wb = np.zeros((128, 8, 128), dtype=bf)
    thwT = np.ascontiguousarray(theta_w.T).reshape(2, 128, I)  # [a, p, i]
    phwT = np.ascontiguousarray(phi_w.T).reshape(2, 128, I)
    gwT = np.ascontiguousarray(g_w.T).reshape(2, 128, I)
    weffT = np.ascontiguousarray(w_eff.T).reshape(I, 2, 128)  # [i, h, c]
    for a in range(2):
        wb[:, 0 + a, :] = thwT[a].astype(bf)
        wb[:, 2 + a, :] = phwT[a].astype(bf)
        wb[:, 4 + a, :] = gwT[a].astype(bf)
        wb[:, 6 + a, :] = weffT[:, a, :].astype(bf)

    fp = np.zeros((128, 4), dtype=np.float32)
    fp[:, 0] = theta_b
    fp[:, 1] = phi_b
    be = b_eff.reshape(2, 128)
    fp[:, 2] = be[0]
    fp[:, 3] = be[1]

    common = {"wb": wb, "fp": fp}
    return [
        {"xt": np.ascontiguousarray(x_this[b]),
         "xtp": np.ascontiguousarray(x_this[b] + b_eff[:, None]),
         "xo": np.ascontiguousarray(x_other[b]), **common}
        for b in range(B)
    ]


def run(inputs: dict, repeat: int = 1, time_it: bool = False):
    nc = _get_built(repeat)
    maps = prep_maps(inputs)
    t0 = time.time()
    res = run_bass_kernel_spmd(nc, maps, list(range(NCORES)))
    wall = time.time() - t0
    out = np.stack([np.asarray(res.results[b]["out"], np.float32)
                    for b in range(B)])
    out = out.reshape(B, C, HH, WW)
    if time_it:
        return out, wall
    return out


def kernel(**inputs) -> np.ndarray:
    return run(inputs)
